# revision 69
# baseline (speedup 1.0000x reference)
"""Trainium2 Bass kernel for nn_Attention (batch=8, seq=1024, dim=1024, 16 heads x 64).

Strategy: pure data parallelism — one batch element per NeuronCore (8 cores),
full weights replicated, zero collectives. Per core:
  LayerNorm (f32 stats) -> qkv matmul in fp32r -> per-head RMS norm ->
  scores^T in fp32r (K=64, 2-head row-packing via partition bases) ->
  exp on ScalarE (no max subtraction needed: |s| <= 64 < 88) ->
  attn@v in bf16 with a ones-column producing softmax denominators ->
  batched reciprocal via exp(-ln) -> normalize -> out-proj (bf16).
All matmul accumulation is fp32 in PSUM. The only ScalarE table set used is
natural_log_exp (exp/ln); squares run on VectorE to avoid table thrashing.
Attention runs in two 8-head chunks so qnT/knT fit in SBUF at fp32r.
"""
import sys

sys.path.insert(0, '/opt/trn_rl_repo')

import numpy as np
import ml_dtypes
import concourse.bass as bass
import concourse.mybir as mybir
import concourse.tile as tile
from concourse import bacc
from concourse.bass_utils import run_bass_kernel_spmd

f32 = mybir.dt.float32
f32r = mybir.dt.float32r
bf16 = mybir.dt.bfloat16
AX = mybir.AxisListType
ALU = mybir.AluOpType
ACTF = mybir.ActivationFunctionType

N = 1024          # tokens per core
D = 1024          # model dim
H = 16            # heads
C = 64            # head dim
NT = N // 128     # token tiles
DT = D // 128     # dim tiles
NP = H // 2       # head pairs

LN_EPS = 1e-5
RMS_EPS = 1e-24


def build():
    nc = bacc.Bacc(None)
    x = nc.declare_dram_parameter("x", [N, D], f32, isOutput=False)
    wqkv = nc.declare_dram_parameter("wqkv", [D, 3 * D], f32r, isOutput=False)
    wout = nc.declare_dram_parameter("wout", [D, D], bf16, isOutput=False)
    g = nc.declare_dram_parameter("g", [1, D], f32, isOutput=False)
    ident = nc.declare_dram_parameter("ident", [128, 128], f32r, isOutput=False)
    selmaps = nc.declare_dram_parameter("selmaps", [4, 8, 128], bf16, isOutput=False)
    out = nc.declare_dram_parameter("out", [N, D], f32, isOutput=True)

    with tile.TileContext(nc) as tc:
        with tc.tile_pool(name="persist", bufs=1) as pp, \
             tc.tile_pool(name="big", bufs=2) as bigp, \
             tc.tile_pool(name="wstream", bufs=6) as wsp, \
             tc.tile_pool(name="stageA", bufs=2) as sta, \
             tc.tile_pool(name="stageB", bufs=2) as stb, \
             tc.tile_pool(name="ptpool", bufs=8) as ptp, \
             tc.tile_pool(name="small", bufs=3) as smp, \
             tc.tile_pool(name="rrep", bufs=1) as rrp, \
             tc.tile_pool(name="smc", bufs=2) as smc, \
             tc.tile_pool(name="ps1024", bufs=2, space="PSUM") as ps1024, \
             tc.tile_pool(name="psb", bufs=2, space="PSUM") as psb, \
             tc.tile_pool(name="pst", bufs=2, space="PSUM") as pst:

            # stream x first — everything else queues behind it on gpsimd
            x_tiles = []
            for tt in range(NT):
                x_sb = sta.tile([128, D], f32, tag="x_t", name=f"x_{tt}")
                nc.gpsimd.dma_start(x_sb[:], x[tt * 128:(tt + 1) * 128, :])
                x_tiles.append(x_sb)
            eps_ln = pp.tile([128, 1], f32, tag="epsln")
            nc.gpsimd.memset(eps_ln[:], LN_EPS)
            id_sb = pp.tile([128, 128], f32r, tag="ident")
            nc.gpsimd.dma_start(id_sb[:], ident[:])
            eps_rms = pp.tile([128, 1], f32, tag="epsrms")
            nc.gpsimd.memset(eps_rms[:], RMS_EPS)
            one_c = pp.tile([128, 1], f32, tag="onec")
            nc.gpsimd.memset(one_c[:], 1.0)
            g_rep = pp.tile([128, D], f32, tag="grep")
            nc.gpsimd.dma_start(g_rep[:], g[0:1, :].to_broadcast((128, D)))
            sel_sb = pp.tile([8, 4, 128], bf16, tag="selmaps")
            nc.sync.dma_start(sel_sb[:], selmaps.rearrange("v k m -> k v m"))

            # Persistent big tensors. xnT / ohn / wout share the 2-slot "big"
            # pool: xnT (slot 0) dies after the last qkv matmul; ohn takes
            # slot 1 during attention; wout reuses slot 0 for the projection.
            xnT = bigp.tile([128, DT, N], f32r, tag="big")        # [d, dt, t]
            qnT = pp.tile([128, 4, N], f32r, tag="qnT")           # [2hx64c, pair%4, t]
            knT = pp.tile([128, 4, N], f32r, tag="knT")
            v_aug = pp.tile([128, NT, H, 66], bf16, tag="vaug")   # [j, jt, h, c|1|pad]
            ohn_box = [None, None]

            def ohn_of(p):
                return (ohn_box[0], p) if p < 4 else (ohn_box[1], p - 4)
            cs_dram = nc.dram_tensor("cs_scratch", [32, 512], f32)
            cs_t3 = pp.tile([8, 512], f32, tag="cst3")
            r_dram = nc.dram_tensor("r_scratch", [32, 512], f32)

            # ---------- Phase A: LayerNorm + transpose x ----------
            for tt in range(NT):
                ts = slice(tt * 128, (tt + 1) * 128)
                x_sb = x_tiles[tt]
                s1 = smp.tile([128, 1], f32, tag="s1")
                nc.vector.tensor_reduce(s1[:], x_sb[:], AX.X, ALU.add)
                xn_t = sta.tile([128, D], f32r, tag="xn_t")
                s2 = smp.tile([128, 1], f32, tag="s2")
                sq_a = rrp.tile([128, N], f32, tag="rpair", name=f"sqa_{tt}")
                nc.scalar.activation(sq_a[:, 0:D], x_sb[:], ACTF.Square,
                                     bias=0.0, scale=1.0, accum_out=s2[:])
                m2 = smp.tile([128, 1], f32, tag="m2")
                nc.vector.tensor_tensor(m2[:], s1[:], s1[:], ALU.mult)
                dvar = smp.tile([128, 1], f32, tag="dvar")
                nc.vector.tensor_scalar(dvar[:], m2[:], -1.0 / D, s2[:], ALU.mult, ALU.add)
                lnv = smp.tile([128, 1], f32, tag="lnv")
                nc.scalar.activation(lnv[:], dvar[:], ACTF.Ln, bias=eps_ln[:], scale=1.0 / D)
                rsig = smp.tile([128, 1], f32, tag="rsig")
                nc.scalar.activation(rsig[:], lnv[:], ACTF.Exp, bias=0.0, scale=-0.5)
                nmr = smp.tile([128, 1], f32, tag="nmr")
                nc.vector.tensor_scalar(nmr[:], s1[:], rsig[:], -1.0 / D, ALU.mult, ALU.mult)
                nc.vector.tensor_scalar(xn_t[:], x_sb[:], rsig[:], nmr[:], ALU.mult, ALU.add)
                for dt_i in range(DT):
                    ps_t = pst.tile([128, 128], f32r, tag="pst")
                    nc.tensor.transpose(ps_t[:], xn_t[:, dt_i * 128:(dt_i + 1) * 128],
                                        id_sb[:])
                    if dt_i % 2 == 0:
                        nc.vector.tensor_copy(xnT[:, dt_i, ts], ps_t[:])
                    else:
                        nc.scalar.copy(xnT[:, dt_i, ts], ps_t[:])

            def do_group(grp):
                """Project one 512-wide column group of wqkv. grp: 0,1=q 2,3=k 4,5=v.
                q/k groups write chunk-local pair slots 0..3 of qnT/knT."""
                kind = grp // 2  # 0=q, 1=k, 2=v
                w_halves = []
                for quarter in range(4):
                    w_sb = wsp.tile([128, DT // 4, 512], f32r, tag="wg",
                                    name=f"w_{grp}_{quarter}")
                    nc.gpsimd.dma_start(
                        w_sb[:], wqkv[quarter * 256:(quarter + 1) * 256,
                                      grp * 512:(grp + 1) * 512]
                        .rearrange("(ko ki) f -> ki ko f", ki=128))
                    w_halves.append(w_sb)
                for tt in range(NT):
                    ts = slice(tt * 128, (tt + 1) * 128)
                    ps_q = psb.tile([128, 512], f32, tag="ps512")
                    for dt_i in range(DT):
                        nc.tensor.matmul(ps_q[:], xnT[:, dt_i, ts],
                                         w_halves[dt_i // 2][:, dt_i % 2, :],
                                         start=(dt_i == 0), stop=(dt_i == DT - 1))
                    if kind == 2:
                        hbase = (grp - 4) * 8
                        nc.scalar.copy(
                            v_aug[:, tt, hbase:hbase + 8, 0:64],
                            ps_q.rearrange("p (h c) -> p h c", c=64))
                    else:
                        q_raw = stb.tile([128, 512], f32, tag="q_raw")
                        nc.vector.tensor_copy(q_raw[:], ps_q[:])
                        sq = stb.tile([128, 512], f32, tag="sq")
                        ss = smp.tile([128, 8], f32, tag="ss")
                        nc.scalar.activation(sq.rearrange("p (h c) -> p h c", c=64),
                                             q_raw.rearrange("p (h c) -> p h c", c=64),
                                             ACTF.Square, bias=0.0, scale=1.0)
                        nc.vector.tensor_reduce(
                            ss[:], sq.rearrange("p (h c) -> p h c", c=64), AX.X, ALU.add)
                        lnss = smp.tile([128, 8], f32, tag="lnss")
                        nc.scalar.activation(lnss[:], ss[:], ACTF.Ln, bias=eps_rms[:], scale=1.0)
                        rsq = smp.tile([128, 8], f32, tag="rsq")
                        nc.scalar.activation(rsq[:], lnss[:], ACTF.Exp, bias=0.0, scale=-0.5)
                        qn_t = stb.tile([128, 512], f32r, tag="qn_t")
                        nc.vector.tensor_tensor(
                            qn_t.rearrange("p (h c) -> p h c", c=64),
                            q_raw.rearrange("p (h c) -> p h c", c=64),
                            rsq[:, :, None].to_broadcast((128, 8, 64)), ALU.mult)
                        if kind == 1:
                            nc.vector.tensor_tensor(
                                qn_t[:], qn_t[:],
                                g_rep[:, (grp - 2) * 512:(grp - 1) * 512], ALU.mult)
                        dstT = qnT if kind == 0 else knT
                        act_help = grp in (0, 2)  # ACT is idle before attention starts
                        for blk in range(4):
                            ps_t2 = pst.tile([128, 128], f32r, tag="pst")
                            nc.tensor.transpose(
                                ps_t2[:], qn_t[:, blk * 128:(blk + 1) * 128], id_sb[:])
                            if act_help and blk % 2 == 1:
                                nc.scalar.copy(dstT[:, blk, ts], ps_t2[:])
                            else:
                                nc.vector.tensor_copy(dstT[:, blk, ts], ps_t2[:])

            def attention_pair(p):
                ohn, po = ohn_of(p)
                pc = p % 4  # chunk-local pair index into qnT/knT
                for hs in range(2):
                    h = 2 * p + hs
                    hp = slice(hs * 64, (hs + 1) * 64)
                    pts = [ptp.tile([128, N], bf16, tag="pT", name=f"pT_{h}_{jt}")
                           for jt in range(NT)]
                    for jt in range(NT):
                        ps_s = ps1024.tile([128, 1024], f32, tag="ps1024")
                        for ih in range(2):
                            nc.tensor.matmul(
                                ps_s[:, ih * 512:(ih + 1) * 512],
                                knT[hp, pc, jt * 128:(jt + 1) * 128],
                                qnT[hp, pc, ih * 512:(ih + 1) * 512],
                                start=True, stop=True)
                        nc.scalar.activation(pts[jt][:], ps_s[:],
                                             ACTF.Exp, bias=0.0, scale=1.0)
                    for ih in range(2):
                        ihs = slice(ih * 512, (ih + 1) * 512)
                        ps_o = psb.tile([128, 512], f32, tag="ps512")
                        for jt in range(NT):
                            nc.tensor.matmul(
                                ps_o[0:65, :], v_aug[:, jt, h, 0:65], pts[jt][:, ihs],
                                start=(jt == 0), stop=(jt == NT - 1))
                        nc.vector.tensor_copy(ohn[hp, po, ihs], ps_o[0:64, :])
                        cs_st = smc.tile([1, 512], f32, tag="cs_st")
                        nc.vector.tensor_copy(cs_st[:], ps_o[64:65, :])
                        if h >= 12:
                            nc.sync.dma_start(
                                cs_t3[2 * h + ih - 24:2 * h + ih - 23, :], cs_st[:])
                        else:
                            nc.sync.dma_start(
                                cs_dram[2 * h + ih:2 * h + ih + 1, :], cs_st[:])

            def finish_pairs(q):
                """Reciprocals + normalize for pair batch q (pairs 2q, 2q+1;
                cs rows 8q..8q+8) — overlapped under later attention / proj."""
                cs_half = pp.tile([8, 512], f32, tag="cshalf", name=f"csh_{q}")
                nc.sync.dma_start(cs_half[:], cs_dram[8 * q:8 * (q + 1), :])
                nc.scalar.activation(cs_half[:], cs_half[:], ACTF.Ln, bias=0.0, scale=1.0)
                r_half = pp.tile([8, 512], f32, tag="rhalf", name=f"rh_{q}")
                nc.scalar.activation(r_half[:], cs_half[:], ACTF.Exp, bias=0.0, scale=-1.0)
                nc.sync.dma_start(r_dram[8 * q:8 * (q + 1), :], r_half[:])
                for p in (2 * q, 2 * q + 1):
                    r_pair = rrp.tile([128, N], f32, tag="rpair", name=f"rp_{p}")
                    for hs in range(2):
                        h = 2 * p + hs
                        for ih in range(2):
                            eng = nc.sync if (hs + ih) % 2 == 0 else nc.gpsimd
                            eng.dma_start(
                                r_pair[hs * 64:(hs + 1) * 64, ih * 512:(ih + 1) * 512],
                                r_dram[2 * h + ih:2 * h + ih + 1, :]
                                .to_broadcast((64, 512)))
                    ohn, po = ohn_of(p)
                    nc.vector.tensor_tensor(ohn[:, po, :], ohn[:, po, :], r_pair[:], ALU.mult)

            # ---------- Phases B/C interleaved in two 8-head chunks ----------
            do_group(4)           # v heads 0..7
            do_group(5)           # v heads 8..15
            nc.vector.tensor_copy(
                v_aug[:, :, :, 64:65].rearrange("p a b o -> p (a b o)"),
                one_c[:].to_broadcast((128, NT * H)))
            do_group(0)           # q heads 0..7
            do_group(2)           # k heads 0..7
            ohn_a = bigp.tile([128, 4, N], bf16, tag="big", name="ohn_a")  # [2hx64c, pair, i]
            ohn_box[0] = ohn_a
            for p in range(4):
                attention_pair(p)
            do_group(1)           # q heads 8..15
            do_group(3)           # k heads 8..15
            # wout in quarters through the (draining) weight-stream pool,
            # on the sync queue so they issue the moment a slot frees
            wout_q = []
            for q in range(4):
                w_sb = wsp.tile([128, 2, D], bf16, tag="wg", name=f"wo_{q}")
                nc.sync.dma_start(
                    w_sb[:], wout[q * 256:(q + 1) * 256, :]
                    .rearrange("(ko ki) d -> ki ko d", ki=128))
                wout_q.append(w_sb)
            finish_pairs(0)       # overlaps chunk-2 attention
            finish_pairs(1)
            ohn_b = bigp.tile([128, 4, N], bf16, tag="big", name="ohn_b")
            ohn_box[1] = ohn_b
            attention_pair(4)
            attention_pair(5)
            finish_pairs(2)
            attention_pair(6)
            attention_pair(7)
            # tail: one DRAM hop, ln in f32, reciprocals to a bf16 leaf, then an
            # 8-row selector matmul broadcasts r across partitions into PSUM
            nc.scalar.activation(cs_t3[:], cs_t3[:], ACTF.Ln, bias=0.0, scale=1.0)
            r_bf = pp.tile([8, 512], bf16, tag="rbf")
            nc.scalar.activation(r_bf[:], cs_t3[:], ACTF.Exp, bias=0.0, scale=-1.0)
            for p in (6, 7):
                ps_r = ps1024.tile([128, 1024], f32, tag="ps1024", name=f"psr_{p}")
                for ih in range(2):
                    nc.tensor.matmul(ps_r[:, ih * 512:(ih + 1) * 512],
                                     sel_sb[:, (p - 6) * 2 + ih, :], r_bf[:],
                                     start=True, stop=True)
                ohn, po = ohn_of(p)
                nc.vector.tensor_tensor(ohn[:, po, :], ohn[:, po, :], ps_r[:], ALU.mult)

            # ---------- Phase D: output proj ----------

            for it in range(NT):
                its = slice(it * 128, (it + 1) * 128)
                for dh in range(2):
                    if (it * 2 + dh) % 2 == 0:
                        ps_f = psb.tile([128, 512], f32, tag="ps512")
                    else:
                        ps_f = ps1024.tile([128, 1024], f32, tag="ps1024",
                                           name=f"psf_{it}_{dh}")[:, 0:512]
                    # two accumulation groups aligned with the ohn_a/ohn_b
                    # tiles: deps are tracked per-tile, so the ohn_a group can
                    # run while chunk 2's softmax chain drains
                    for p in range(4):
                        ohn, po = ohn_of(p)
                        nc.tensor.matmul(
                            ps_f[:], ohn[:, po, its],
                            wout_q[p // 2][:, p % 2, dh * 512:(dh + 1) * 512],
                            start=(p == 0), stop=(p == 3))
                    for p in range(4, 8):
                        ohn, po = ohn_of(p)
                        nc.tensor.matmul(
                            ps_f[:], ohn[:, po, its],
                            wout_q[p // 2][:, p % 2, dh * 512:(dh + 1) * 512],
                            start=False, stop=(p == 7), skip_group_check=True)
                    o_sb = stb.tile([128, 512], f32, tag="o_sb")
                    nc.vector.tensor_copy(o_sb[:], ps_f[:])
                    nc.sync.dma_start(out[its, dh * 512:(dh + 1) * 512], o_sb[:])
    return nc


_NC_CACHE = None


def _patch_act_tables():
    """Steer bacc's greedy act-table-set selection to natural_log_exp_and_others
    for every function this kernel uses (exp/ln/square/copy/identity), by
    hiding those functions from all earlier sets. Set order (and thus the
    act_func_set_id each load emits) is unchanged, so the runtime tables are
    correct — but all our activations resolve to one co-resident set and the
    kernel performs a single table load instead of thrashing."""
    import collections
    import concourse.bacc as _bacc
    import concourse.hw_specs as _hw
    orig = getattr(_hw.get_activation_tables, '__wrapped_orig__', _hw.get_activation_tables)

    def patched(arch):
        d = orig(arch)
        key = "natural_log_exp_and_others"
        if key not in d:
            return d
        mine = d[key]
        hidden = {f for f in mine}
        nd = collections.OrderedDict()
        for k, v in d.items():
            if k == key:
                nd[k] = v
            else:
                nd[k] = v - hidden
        return nd
    patched.__wrapped_orig__ = orig
    _hw.get_activation_tables = patched
    _bacc.get_activation_tables = patched


def _get_nc():
    global _NC_CACHE
    if _NC_CACHE is None:
        _patch_act_tables()
        nc = build()
        nc.finalize()
        _NC_CACHE = nc
    return _NC_CACHE


def kernel(x, ln_gamma, q_gamma, k_gamma, w_qkv, w_out):
    x = np.asarray(x, dtype=np.float32)
    ln_gamma = np.asarray(ln_gamma, dtype=np.float32)
    q_gamma = np.asarray(q_gamma, dtype=np.float32).reshape(H, C)
    k_gamma = np.asarray(k_gamma, dtype=np.float32).reshape(H, C)
    w_qkv = np.asarray(w_qkv, dtype=np.float32)
    w_out = np.asarray(w_out, dtype=np.float32)

    wqkv_eff = np.ascontiguousarray(ln_gamma[:, None] * w_qkv, dtype=np.float32)
    wout_bf = w_out.astype(ml_dtypes.bfloat16)
    g = (64.0 * q_gamma * k_gamma).reshape(1, D).astype(np.float32)
    ident = np.eye(128, dtype=np.float32)
    selmaps = np.zeros((4, 8, 128), dtype=np.float32)
    for v in range(4):
        pp_, ih = v // 2, v % 2
        for m in range(128):
            selmaps[v, 4 * pp_ + 2 * (m // 64) + ih, m] = 1.0
    selmaps = selmaps.astype(ml_dtypes.bfloat16)

    nc = _get_nc()
    in_maps = [
        {"x": np.ascontiguousarray(x[i]), "wqkv": wqkv_eff, "wout": wout_bf,
         "g": g, "ident": ident, "selmaps": selmaps}
        for i in range(8)
    ]
    res = run_bass_kernel_spmd(nc, in_maps, core_ids=list(range(8)))
    return np.stack([res.results[i]["out"] for i in range(8)], axis=0)


# revision 70
# speedup vs baseline: 1.0006x; 1.0006x over previous
"""Trainium2 Bass kernel for nn_Attention (batch=8, seq=1024, dim=1024, 16 heads x 64).

Strategy: pure data parallelism — one batch element per NeuronCore (8 cores),
full weights replicated, zero collectives. Per core:
  LayerNorm (f32 stats) -> qkv matmul in fp32r -> per-head RMS norm ->
  scores^T in fp32r (K=64, 2-head row-packing via partition bases) ->
  exp on ScalarE (no max subtraction needed: |s| <= 64 < 88) ->
  attn@v in bf16 with a ones-column producing softmax denominators ->
  batched reciprocal via exp(-ln) -> normalize -> out-proj (bf16).
All matmul accumulation is fp32 in PSUM. The only ScalarE table set used is
natural_log_exp (exp/ln); squares run on VectorE to avoid table thrashing.
Attention runs in two 8-head chunks so qnT/knT fit in SBUF at fp32r.
"""
import sys

sys.path.insert(0, '/opt/trn_rl_repo')

import numpy as np
import ml_dtypes
import concourse.bass as bass
import concourse.mybir as mybir
import concourse.tile as tile
from concourse import bacc
from concourse.bass_utils import run_bass_kernel_spmd

f32 = mybir.dt.float32
f32r = mybir.dt.float32r
bf16 = mybir.dt.bfloat16
AX = mybir.AxisListType
ALU = mybir.AluOpType
ACTF = mybir.ActivationFunctionType

N = 1024          # tokens per core
D = 1024          # model dim
H = 16            # heads
C = 64            # head dim
NT = N // 128     # token tiles
DT = D // 128     # dim tiles
NP = H // 2       # head pairs

LN_EPS = 1e-5
RMS_EPS = 1e-24


def build():
    nc = bacc.Bacc(None)
    x = nc.declare_dram_parameter("x", [N, D], f32, isOutput=False)
    wqkv = nc.declare_dram_parameter("wqkv", [D, 3 * D], f32r, isOutput=False)
    wout = nc.declare_dram_parameter("wout", [D, D], bf16, isOutput=False)
    g = nc.declare_dram_parameter("g", [1, D], f32, isOutput=False)
    ident = nc.declare_dram_parameter("ident", [128, 128], f32r, isOutput=False)
    selmaps = nc.declare_dram_parameter("selmaps", [4, 8, 128], bf16, isOutput=False)
    out = nc.declare_dram_parameter("out", [N, D], f32, isOutput=True)

    with tile.TileContext(nc) as tc:
        with tc.tile_pool(name="persist", bufs=1) as pp, \
             tc.tile_pool(name="big", bufs=2) as bigp, \
             tc.tile_pool(name="wstream", bufs=6) as wsp, \
             tc.tile_pool(name="stageA", bufs=2) as sta, \
             tc.tile_pool(name="stageB", bufs=2) as stb, \
             tc.tile_pool(name="ptpool", bufs=8) as ptp, \
             tc.tile_pool(name="small", bufs=3) as smp, \
             tc.tile_pool(name="rrep", bufs=1) as rrp, \
             tc.tile_pool(name="smc", bufs=2) as smc, \
             tc.tile_pool(name="ps1024", bufs=2, space="PSUM") as ps1024, \
             tc.tile_pool(name="psb", bufs=2, space="PSUM") as psb, \
             tc.tile_pool(name="pst", bufs=2, space="PSUM") as pst:

            # stream x first — everything else queues behind it on gpsimd
            x_tiles = []
            for tt in range(NT):
                x_sb = sta.tile([128, D], f32, tag="x_t", name=f"x_{tt}")
                nc.gpsimd.dma_start(x_sb[:], x[tt * 128:(tt + 1) * 128, :])
                x_tiles.append(x_sb)
            eps_ln = pp.tile([128, 1], f32, tag="epsln")
            nc.gpsimd.memset(eps_ln[:], LN_EPS)
            id_sb = pp.tile([128, 128], f32r, tag="ident")
            nc.gpsimd.dma_start(id_sb[:], ident[:])
            eps_rms = pp.tile([128, 1], f32, tag="epsrms")
            nc.gpsimd.memset(eps_rms[:], RMS_EPS)
            one_c = pp.tile([128, 1], f32, tag="onec")
            nc.gpsimd.memset(one_c[:], 1.0)
            g_rep = pp.tile([128, D], f32, tag="grep")
            nc.gpsimd.dma_start(g_rep[:], g[0:1, :].to_broadcast((128, D)))
            sel_sb = pp.tile([8, 4, 128], bf16, tag="selmaps")
            nc.sync.dma_start(sel_sb[:], selmaps.rearrange("v k m -> k v m"))

            # Persistent big tensors. xnT / ohn / wout share the 2-slot "big"
            # pool: xnT (slot 0) dies after the last qkv matmul; ohn takes
            # slot 1 during attention; wout reuses slot 0 for the projection.
            xnT = bigp.tile([128, DT, N], f32r, tag="big")        # [d, dt, t]
            qnT = pp.tile([128, 4, N], f32r, tag="qnT")           # [2hx64c, pair%4, t]
            knT = pp.tile([128, 4, N], f32r, tag="knT")
            v_aug = pp.tile([128, NT, H, 66], bf16, tag="vaug")   # [j, jt, h, c|1|pad]
            ohn_box = [None, None]

            def ohn_of(p):
                return (ohn_box[0], p) if p < 4 else (ohn_box[1], p - 4)
            cs_dram = nc.dram_tensor("cs_scratch", [32, 512], f32)
            cs_t3 = pp.tile([8, 512], f32, tag="cst3")
            r_dram = nc.dram_tensor("r_scratch", [32, 512], f32)

            # ---------- Phase A: LayerNorm + transpose x ----------
            for tt in range(NT):
                ts = slice(tt * 128, (tt + 1) * 128)
                x_sb = x_tiles[tt]
                s1 = smp.tile([128, 1], f32, tag="s1")
                nc.vector.tensor_reduce(s1[:], x_sb[:], AX.X, ALU.add)
                xn_t = sta.tile([128, D], f32r, tag="xn_t")
                s2 = smp.tile([128, 1], f32, tag="s2")
                sq_a = rrp.tile([128, N], f32, tag="rpair", name=f"sqa_{tt}")
                nc.scalar.activation(sq_a[:, 0:D], x_sb[:], ACTF.Square,
                                     bias=0.0, scale=1.0, accum_out=s2[:])
                m2 = smp.tile([128, 1], f32, tag="m2")
                nc.vector.tensor_tensor(m2[:], s1[:], s1[:], ALU.mult)
                dvar = smp.tile([128, 1], f32, tag="dvar")
                nc.vector.tensor_scalar(dvar[:], m2[:], -1.0 / D, s2[:], ALU.mult, ALU.add)
                lnv = smp.tile([128, 1], f32, tag="lnv")
                nc.scalar.activation(lnv[:], dvar[:], ACTF.Ln, bias=eps_ln[:], scale=1.0 / D)
                rsig = smp.tile([128, 1], f32, tag="rsig")
                nc.scalar.activation(rsig[:], lnv[:], ACTF.Exp, bias=0.0, scale=-0.5)
                nmr = smp.tile([128, 1], f32, tag="nmr")
                nc.vector.tensor_scalar(nmr[:], s1[:], rsig[:], -1.0 / D, ALU.mult, ALU.mult)
                nc.vector.tensor_scalar(xn_t[:, 0:512], x_sb[:, 0:512], rsig[:], nmr[:],
                                        ALU.mult, ALU.add)
                nc.vector.tensor_scalar(xn_t[:, 512:D], x_sb[:, 512:D], rsig[:], nmr[:],
                                        ALU.mult, ALU.add)
                for dt_i in range(DT):
                    ps_t = pst.tile([128, 128], f32r, tag="pst")
                    nc.tensor.transpose(ps_t[:], xn_t[:, dt_i * 128:(dt_i + 1) * 128],
                                        id_sb[:])
                    if dt_i % 2 == 0:
                        nc.vector.tensor_copy(xnT[:, dt_i, ts], ps_t[:])
                    else:
                        nc.scalar.copy(xnT[:, dt_i, ts], ps_t[:])

            def do_group(grp):
                """Project one 512-wide column group of wqkv. grp: 0,1=q 2,3=k 4,5=v.
                q/k groups write chunk-local pair slots 0..3 of qnT/knT."""
                kind = grp // 2  # 0=q, 1=k, 2=v
                w_halves = []
                for quarter in range(4):
                    w_sb = wsp.tile([128, DT // 4, 512], f32r, tag="wg",
                                    name=f"w_{grp}_{quarter}")
                    nc.gpsimd.dma_start(
                        w_sb[:], wqkv[quarter * 256:(quarter + 1) * 256,
                                      grp * 512:(grp + 1) * 512]
                        .rearrange("(ko ki) f -> ki ko f", ki=128))
                    w_halves.append(w_sb)
                for tt in range(NT):
                    ts = slice(tt * 128, (tt + 1) * 128)
                    ps_q = psb.tile([128, 512], f32, tag="ps512")
                    for dt_i in range(DT):
                        nc.tensor.matmul(ps_q[:], xnT[:, dt_i, ts],
                                         w_halves[dt_i // 2][:, dt_i % 2, :],
                                         start=(dt_i == 0), stop=(dt_i == DT - 1))
                    if kind == 2:
                        hbase = (grp - 4) * 8
                        nc.scalar.copy(
                            v_aug[:, tt, hbase:hbase + 8, 0:64],
                            ps_q.rearrange("p (h c) -> p h c", c=64))
                    else:
                        q_raw = stb.tile([128, 512], f32, tag="q_raw")
                        nc.vector.tensor_copy(q_raw[:], ps_q[:])
                        sq = stb.tile([128, 512], f32, tag="sq")
                        ss = smp.tile([128, 8], f32, tag="ss")
                        nc.scalar.activation(sq.rearrange("p (h c) -> p h c", c=64),
                                             q_raw.rearrange("p (h c) -> p h c", c=64),
                                             ACTF.Square, bias=0.0, scale=1.0)
                        nc.vector.tensor_reduce(
                            ss[:], sq.rearrange("p (h c) -> p h c", c=64), AX.X, ALU.add)
                        lnss = smp.tile([128, 8], f32, tag="lnss")
                        nc.scalar.activation(lnss[:], ss[:], ACTF.Ln, bias=eps_rms[:], scale=1.0)
                        rsq = smp.tile([128, 8], f32, tag="rsq")
                        nc.scalar.activation(rsq[:], lnss[:], ACTF.Exp, bias=0.0, scale=-0.5)
                        qn_t = stb.tile([128, 512], f32r, tag="qn_t")
                        nc.vector.tensor_tensor(
                            qn_t.rearrange("p (h c) -> p h c", c=64),
                            q_raw.rearrange("p (h c) -> p h c", c=64),
                            rsq[:, :, None].to_broadcast((128, 8, 64)), ALU.mult)
                        if kind == 1:
                            nc.vector.tensor_tensor(
                                qn_t[:], qn_t[:],
                                g_rep[:, (grp - 2) * 512:(grp - 1) * 512], ALU.mult)
                        dstT = qnT if kind == 0 else knT
                        act_help = grp in (0, 2)  # ACT is idle before attention starts
                        for blk in range(4):
                            ps_t2 = pst.tile([128, 128], f32r, tag="pst")
                            nc.tensor.transpose(
                                ps_t2[:], qn_t[:, blk * 128:(blk + 1) * 128], id_sb[:])
                            if act_help and blk % 2 == 1:
                                nc.scalar.copy(dstT[:, blk, ts], ps_t2[:])
                            else:
                                nc.vector.tensor_copy(dstT[:, blk, ts], ps_t2[:])

            def attention_pair(p):
                ohn, po = ohn_of(p)
                pc = p % 4  # chunk-local pair index into qnT/knT
                for hs in range(2):
                    h = 2 * p + hs
                    hp = slice(hs * 64, (hs + 1) * 64)
                    pts = [ptp.tile([128, N], bf16, tag="pT", name=f"pT_{h}_{jt}")
                           for jt in range(NT)]
                    for jt in range(NT):
                        ps_s = ps1024.tile([128, 1024], f32, tag="ps1024")
                        for ih in range(2):
                            nc.tensor.matmul(
                                ps_s[:, ih * 512:(ih + 1) * 512],
                                knT[hp, pc, jt * 128:(jt + 1) * 128],
                                qnT[hp, pc, ih * 512:(ih + 1) * 512],
                                start=True, stop=True)
                        nc.scalar.activation(pts[jt][:], ps_s[:],
                                             ACTF.Exp, bias=0.0, scale=1.0)
                    for ih in range(2):
                        ihs = slice(ih * 512, (ih + 1) * 512)
                        ps_o = psb.tile([128, 512], f32, tag="ps512")
                        for jt in range(NT):
                            nc.tensor.matmul(
                                ps_o[0:65, :], v_aug[:, jt, h, 0:65], pts[jt][:, ihs],
                                start=(jt == 0), stop=(jt == NT - 1))
                        nc.vector.tensor_copy(ohn[hp, po, ihs], ps_o[0:64, :])
                        cs_st = smc.tile([1, 512], f32, tag="cs_st")
                        nc.vector.tensor_copy(cs_st[:], ps_o[64:65, :])
                        if h >= 12:
                            nc.sync.dma_start(
                                cs_t3[2 * h + ih - 24:2 * h + ih - 23, :], cs_st[:])
                        else:
                            nc.sync.dma_start(
                                cs_dram[2 * h + ih:2 * h + ih + 1, :], cs_st[:])

            def finish_pairs(q):
                """Reciprocals + normalize for pair batch q (pairs 2q, 2q+1;
                cs rows 8q..8q+8) — overlapped under later attention / proj."""
                cs_half = pp.tile([8, 512], f32, tag="cshalf", name=f"csh_{q}")
                nc.sync.dma_start(cs_half[:], cs_dram[8 * q:8 * (q + 1), :])
                nc.scalar.activation(cs_half[:], cs_half[:], ACTF.Ln, bias=0.0, scale=1.0)
                r_half = pp.tile([8, 512], f32, tag="rhalf", name=f"rh_{q}")
                nc.scalar.activation(r_half[:], cs_half[:], ACTF.Exp, bias=0.0, scale=-1.0)
                nc.sync.dma_start(r_dram[8 * q:8 * (q + 1), :], r_half[:])
                for p in (2 * q, 2 * q + 1):
                    r_pair = rrp.tile([128, N], f32, tag="rpair", name=f"rp_{p}")
                    for hs in range(2):
                        h = 2 * p + hs
                        for ih in range(2):
                            eng = nc.sync if (hs + ih) % 2 == 0 else nc.gpsimd
                            eng.dma_start(
                                r_pair[hs * 64:(hs + 1) * 64, ih * 512:(ih + 1) * 512],
                                r_dram[2 * h + ih:2 * h + ih + 1, :]
                                .to_broadcast((64, 512)))
                    ohn, po = ohn_of(p)
                    nc.vector.tensor_tensor(ohn[:, po, :], ohn[:, po, :], r_pair[:], ALU.mult)

            # ---------- Phases B/C interleaved in two 8-head chunks ----------
            do_group(4)           # v heads 0..7
            do_group(5)           # v heads 8..15
            nc.vector.tensor_copy(
                v_aug[:, :, :, 64:65].rearrange("p a b o -> p (a b o)"),
                one_c[:].to_broadcast((128, NT * H)))
            do_group(0)           # q heads 0..7
            do_group(2)           # k heads 0..7
            ohn_a = bigp.tile([128, 4, N], bf16, tag="big", name="ohn_a")  # [2hx64c, pair, i]
            ohn_box[0] = ohn_a
            for p in range(4):
                attention_pair(p)
            do_group(1)           # q heads 8..15
            do_group(3)           # k heads 8..15
            # wout in quarters through the (draining) weight-stream pool,
            # on the sync queue so they issue the moment a slot frees
            wout_q = []
            for q in range(4):
                w_sb = wsp.tile([128, 2, D], bf16, tag="wg", name=f"wo_{q}")
                nc.sync.dma_start(
                    w_sb[:], wout[q * 256:(q + 1) * 256, :]
                    .rearrange("(ko ki) d -> ki ko d", ki=128))
                wout_q.append(w_sb)
            finish_pairs(0)       # overlaps chunk-2 attention
            finish_pairs(1)
            ohn_b = bigp.tile([128, 4, N], bf16, tag="big", name="ohn_b")
            ohn_box[1] = ohn_b
            attention_pair(4)
            attention_pair(5)
            finish_pairs(2)
            attention_pair(6)
            attention_pair(7)
            # tail: one DRAM hop, ln in f32, reciprocals to a bf16 leaf, then an
            # 8-row selector matmul broadcasts r across partitions into PSUM
            nc.scalar.activation(cs_t3[:], cs_t3[:], ACTF.Ln, bias=0.0, scale=1.0)
            r_bf = pp.tile([8, 512], bf16, tag="rbf")
            nc.scalar.activation(r_bf[:], cs_t3[:], ACTF.Exp, bias=0.0, scale=-1.0)
            for p in (6, 7):
                ps_r = ps1024.tile([128, 1024], f32, tag="ps1024", name=f"psr_{p}")
                for ih in range(2):
                    nc.tensor.matmul(ps_r[:, ih * 512:(ih + 1) * 512],
                                     sel_sb[:, (p - 6) * 2 + ih, :], r_bf[:],
                                     start=True, stop=True)
                ohn, po = ohn_of(p)
                nc.vector.tensor_tensor(ohn[:, po, :], ohn[:, po, :], ps_r[:], ALU.mult)

            # ---------- Phase D: output proj ----------

            for it in range(NT):
                its = slice(it * 128, (it + 1) * 128)
                for dh in range(2):
                    if (it * 2 + dh) % 2 == 0:
                        ps_f = psb.tile([128, 512], f32, tag="ps512")
                    else:
                        ps_f = ps1024.tile([128, 1024], f32, tag="ps1024",
                                           name=f"psf_{it}_{dh}")[:, 0:512]
                    # two accumulation groups aligned with the ohn_a/ohn_b
                    # tiles: deps are tracked per-tile, so the ohn_a group can
                    # run while chunk 2's softmax chain drains
                    for p in range(4):
                        ohn, po = ohn_of(p)
                        nc.tensor.matmul(
                            ps_f[:], ohn[:, po, its],
                            wout_q[p // 2][:, p % 2, dh * 512:(dh + 1) * 512],
                            start=(p == 0), stop=(p == 3))
                    for p in range(4, 8):
                        ohn, po = ohn_of(p)
                        nc.tensor.matmul(
                            ps_f[:], ohn[:, po, its],
                            wout_q[p // 2][:, p % 2, dh * 512:(dh + 1) * 512],
                            start=False, stop=(p == 7), skip_group_check=True)
                    o_sb = stb.tile([128, 512], f32, tag="o_sb")
                    nc.vector.tensor_copy(o_sb[:], ps_f[:])
                    nc.sync.dma_start(out[its, dh * 512:(dh + 1) * 512], o_sb[:])
    return nc


_NC_CACHE = None


def _patch_act_tables():
    """Steer bacc's greedy act-table-set selection to natural_log_exp_and_others
    for every function this kernel uses (exp/ln/square/copy/identity), by
    hiding those functions from all earlier sets. Set order (and thus the
    act_func_set_id each load emits) is unchanged, so the runtime tables are
    correct — but all our activations resolve to one co-resident set and the
    kernel performs a single table load instead of thrashing."""
    import collections
    import concourse.bacc as _bacc
    import concourse.hw_specs as _hw
    orig = getattr(_hw.get_activation_tables, '__wrapped_orig__', _hw.get_activation_tables)

    def patched(arch):
        d = orig(arch)
        key = "natural_log_exp_and_others"
        if key not in d:
            return d
        mine = d[key]
        hidden = {f for f in mine}
        nd = collections.OrderedDict()
        for k, v in d.items():
            if k == key:
                nd[k] = v
            else:
                nd[k] = v - hidden
        return nd
    patched.__wrapped_orig__ = orig
    _hw.get_activation_tables = patched
    _bacc.get_activation_tables = patched


def _get_nc():
    global _NC_CACHE
    if _NC_CACHE is None:
        _patch_act_tables()
        nc = build()
        nc.finalize()
        _NC_CACHE = nc
    return _NC_CACHE


def kernel(x, ln_gamma, q_gamma, k_gamma, w_qkv, w_out):
    x = np.asarray(x, dtype=np.float32)
    ln_gamma = np.asarray(ln_gamma, dtype=np.float32)
    q_gamma = np.asarray(q_gamma, dtype=np.float32).reshape(H, C)
    k_gamma = np.asarray(k_gamma, dtype=np.float32).reshape(H, C)
    w_qkv = np.asarray(w_qkv, dtype=np.float32)
    w_out = np.asarray(w_out, dtype=np.float32)

    wqkv_eff = np.ascontiguousarray(ln_gamma[:, None] * w_qkv, dtype=np.float32)
    wout_bf = w_out.astype(ml_dtypes.bfloat16)
    g = (64.0 * q_gamma * k_gamma).reshape(1, D).astype(np.float32)
    ident = np.eye(128, dtype=np.float32)
    selmaps = np.zeros((4, 8, 128), dtype=np.float32)
    for v in range(4):
        pp_, ih = v // 2, v % 2
        for m in range(128):
            selmaps[v, 4 * pp_ + 2 * (m // 64) + ih, m] = 1.0
    selmaps = selmaps.astype(ml_dtypes.bfloat16)

    nc = _get_nc()
    in_maps = [
        {"x": np.ascontiguousarray(x[i]), "wqkv": wqkv_eff, "wout": wout_bf,
         "g": g, "ident": ident, "selmaps": selmaps}
        for i in range(8)
    ]
    res = run_bass_kernel_spmd(nc, in_maps, core_ids=list(range(8)))
    return np.stack([res.results[i]["out"] for i in range(8)], axis=0)
